# revision 1
# baseline (speedup 1.0000x reference)
"""ConcatNonLocalBlock kernel for 8x Trainium2 NeuronCores.

Math: the reference's attention matrix attn[b,i,j] = s[b,i]/n is constant
along j, so the whole block collapses to a rank-2 correction of x:

    out[b,c,i] = x[b,c,i] + bexp[c] + s[b,i] * uu[b,c]
    s[b,i]  = ReLU(wS . x[b,:,i] + bS)          wS = Wq^T wq_c + Wk^T wk_c
    uu[b,:] = (Wexp Wv) @ xsum[b] / n + Wexp bv  xsum[b,c] = sum_i x[b,c,i]

Sharding: data-parallel over batch, one sample per core (B=8, 8 cores).

Constraint shaping: PE matmul instructions can carry at most ONE sync wait
(LDWEIGHTS slot), so all small weights are packed into a single DRAM tensor
loaded by one DMA, PE observes it via one tiny "observer" matmul, x is
loaded with one whole-tile DMA per partition tile, and the [uu; bexp] lhsT
is produced by a single ACT copy.
"""

import os
import sys

import numpy as np

sys.path.insert(0, "/opt/trn_rl_repo")

import concourse.bass as bass
import concourse.tile as tile
from concourse import mybir
from concourse.bass_utils import run_bass_kernel_spmd

B, C, H, W = 8, 256, 56, 56
N = H * W  # 3136
E = C // 2  # 128
P = 128  # SBUF partitions
NT = C // P  # 2 partition tiles
CHUNK = 512
CHUNKS = [(c0, min(CHUNK, N - c0)) for c0 in range(0, N, CHUNK)]

# packed "smalls" layout: [128, SM_F] f32 (bf16 payloads packed two-per-word,
# read on device via AP.bitcast(bfloat16))
SM_WSBF = 0  # cols 0..1 (bf16): col t halves = [ws[t*128+p], 0]
SM_WVETBF = 2  # cols 2..257 (bf16): t*128+j halves = Wve[2j+h, t*128+k]/N
SM_ONEBF = 258  # (bf16) [0, 258] low half = 1.0
SM_WBVBF = 259  # cols 259..386 (bf16): [0, 259+j] halves = (Wexp@bv)[2j+h]
SM_BS = 387  # f32 [0, 387] = bS (ReLU bias)
SM_BEXPC = 388  # f32 cols 388..389: col t = bexp[t*128+p] (per-partition scalar)
SM_F = 390

F32 = mybir.dt.float32

LAST_RESULTS = None  # BassKernelResults from the most recent run (for test.py)

_prog_cache = {}


class _SplitDrainTC(tile.TileContext):
    """TileContext whose exit drain splits its sem waits across single-wait
    NoOps: this walrus build rejects any instruction carrying more than one
    sync wait, and the stock kernel-tail drain collects the whole residual
    vector clock onto one Drain."""

    def _drain_and_barrier(self, tick_clock, wait_clock):
        from concourse.vector_clock import ScopedClock

        drain_inst = self.nc.sync.drain()
        wait_clock.add_sem_waits(
            drain_inst.ins, ScopedClock({None: tick_clock.global_clock})
        )
        si = drain_inst.ins.sync_info
        if si is not None and len(si.on_wait) > 1:
            waits = list(si.on_wait)
            drain_inst.ins.sync_info = mybir.SyncInfo(
                on_wait=[], on_update=list(si.on_update)
            )
            for w in waits:
                nop = self.nc.sync.nop()
                nop.ins.sync_info = mybir.SyncInfo(on_wait=[w], on_update=[])

        self.nc.all_engine_barrier()
        assert self.sems is not None
        popped = self.nc._tile_sem_poison_stack.pop()
        assert popped is self._sem_poison
        self.nc.clear_and_free_semaphores(list(self.sems.allocated().values()))
        self.nc.all_engine_barrier()


def _build():
    nc = bass.Bass()
    x_in = nc.dram_tensor("xb", [C, N], F32, kind="ExternalInput")
    sm_in = nc.dram_tensor("smalls", [P, SM_F], F32, kind="ExternalInput")
    out = nc.dram_tensor("out", [C, N], F32, kind="ExternalOutput")

    with _SplitDrainTC(nc) as tc:
        with (
            tc.tile_pool(name="persist", bufs=1) as persist,
            tc.tile_pool(name="ps_s", bufs=2, space="PSUM") as ps_s,
            tc.tile_pool(name="ps_u", bufs=1, space="PSUM") as ps_u,
            tc.tile_pool(name="ps_o", bufs=4, space="PSUM") as ps_o,
        ):
            BF16 = mybir.dt.bfloat16
            sm = persist.tile([P, SM_F], F32, tag="sm")
            nc.gpsimd.dma_start(out=sm, in_=sm_in[:, :])

            def smbf(p0, p1, c0, c1):  # bf16 view of smalls cols [c0:c1)
                return sm[p0:p1, c0:c1].bitcast(BF16)

            s_bf = persist.tile([1, N], BF16, tag="s_bf")
            xsum_sb = persist.tile([P, NT], F32, tag="xsum_sb")
            xsum_bf = persist.tile([P, NT], BF16, tag="xsum_bf")
            uu_bf = persist.tile([1, C], BF16, tag="uu_bf")

            # Engine observers for the smalls DMA (so later ops carry only
            # their one data wait).
            dve_scratch = persist.tile([1, 1], F32, tag="dve_scratch")
            nc.vector.tensor_copy(out=dve_scratch, in_=sm[0:1, SM_BS : SM_BS + 1])
            act_scratch = persist.tile([1, 1], F32, tag="act_scratch")
            nc.scalar.copy(out=act_scratch, in_=sm[0:1, SM_BS : SM_BS + 1])

            # Both partition tiles of x in ONE SBUF tile via ONE SWDGE DMA.
            xall = persist.tile([P, NT, N], F32, tag="xall")
            nc.gpsimd.dma_start(
                out=xall[:, :, :],
                in_=x_in[:, :].rearrange("(t p) n -> p t n", p=P),
            )
            x_sb = [xall[:, t, :] for t in range(NT)]

            # bf16 copy of x for the PE matmuls (ACT)
            xbf = persist.tile([P, NT, N], BF16, tag="xbf")
            for t in range(NT):
                nc.scalar.copy(out=xbf[:, t, :], in_=x_sb[t])

            # Observer matmul: the single PE wait on the smalls DMA. Reuses
            # the ps_u bank; later uu matmuls reset it with start=True.
            uu_psum = ps_u.tile([1, C], F32, tag="uu_psum")
            nc.tensor.matmul(
                uu_psum[0:1, 0:1],
                lhsT=smbf(0, P, SM_WSBF, SM_WSBF + 1)[:, 0:1],
                rhs=smbf(0, P, SM_WSBF, SM_WSBF + 1)[:, 0:1],
                start=True,
                stop=True,
            )

            # s = relu(wS . x + bS): K-accumulated bf16 matvec per chunk, relu
            # on ACT (psum f32 in, bf16 out)
            for ci, (c0, w) in enumerate(CHUNKS):
                s_psum = ps_s.tile([1, CHUNK], F32, tag="s_psum")
                for t in range(NT):
                    nc.tensor.matmul(
                        s_psum[:, :w],
                        lhsT=smbf(0, P, SM_WSBF + t, SM_WSBF + t + 1)[:, 0:1],
                        rhs=xbf[:, t, c0 : c0 + w],
                        start=(t == 0),
                        stop=(t == NT - 1),
                    )
                nc.scalar.activation(
                    out=s_bf[0:1, c0 : c0 + w],
                    in_=s_psum[0:1, :w],
                    func=mybir.ActivationFunctionType.Relu,
                    bias=sm[0:1, SM_BS : SM_BS + 1],
                    scale=1.0,
                )

            # row sums of x (f32), then bf16 for use as matmul weights
            for t in range(NT):
                nc.vector.reduce_sum(
                    out=xsum_sb[:, t : t + 1],
                    in_=x_sb[t][:, :],
                    axis=mybir.AxisListType.X,
                )
            nc.vector.tensor_copy(out=xsum_bf[:, :], in_=xsum_sb[:, :])

            # uu = Wve/N @ xsum + Wexp bv  (row layout [1, C], bf16 matmuls)
            nc.tensor.matmul(
                uu_psum[:, :],
                lhsT=smbf(0, 1, SM_ONEBF, SM_ONEBF + 1)[:, 0:1],
                rhs=smbf(0, 1, SM_WBVBF, SM_WBVBF + C // 2),
                start=True,
                stop=False,
                skip_group_check=True,
            )
            for t in range(NT):
                nc.tensor.matmul(
                    uu_psum[:, :],
                    lhsT=xsum_bf[:, t : t + 1],
                    rhs=smbf(0, P, SM_WVETBF + t * P, SM_WVETBF + (t + 1) * P),
                    start=False,
                    stop=(t == NT - 1),
                    skip_group_check=True,
                )
            nc.scalar.copy(out=uu_bf[:, :], in_=uu_psum[:, :])

            # out = x + uu (x) s + bexp: K=1 bf16 outer product into psum;
            # ACT copies psum into a fresh slice of the output tile (so the
            # PSUM-bank WAR partner is ACT, merging with the matmul's ACT data
            # wait); DVE then adds x + bexp_col in place, bexp f32-exact.
            o_sb = persist.tile([P, NT, N], F32, tag="o_sb")
            for t in range(NT):
                for ci, (c0, w) in enumerate(CHUNKS):
                    o_psum = ps_o.tile([P, CHUNK], F32, tag="o_psum")
                    nc.tensor.matmul(
                        o_psum[:, :w],
                        lhsT=uu_bf[0:1, t * P : (t + 1) * P],
                        rhs=s_bf[0:1, c0 : c0 + w],
                        start=True,
                        stop=True,
                    )
                    nc.scalar.copy(
                        out=o_sb[:, t, c0 : c0 + w],
                        in_=o_psum[:, :w],
                    )
                    nc.vector.scalar_tensor_tensor(
                        out=o_sb[:, t, c0 : c0 + w],
                        in0=o_sb[:, t, c0 : c0 + w],
                        scalar=sm[:, SM_BEXPC + t : SM_BEXPC + t + 1],
                        in1=x_sb[t][:, c0 : c0 + w],
                        op0=mybir.AluOpType.add,
                        op1=mybir.AluOpType.add,
                    )
            for t in range(NT):
                nc.sync.dma_start(
                    out=out[t * P : (t + 1) * P, :],
                    in_=o_sb[:, t, :],
                )
    return nc


def _pack_smalls(Wq, bq, Wk, bk, Wv, bv, Wcat, Wexp, bexp):
    import ml_dtypes

    f32 = np.float32
    wq_c, wk_c = Wcat[0, :E], Wcat[0, E:]
    wS = (Wq.T @ wq_c + Wk.T @ wk_c).astype(f32)  # [C]
    bS = f32(wq_c @ bq + wk_c @ bk)
    Wve = (Wexp @ Wv).astype(f32)  # [C, C]
    wvet = (Wve.T / f32(N)).astype(f32)  # [C, C] : [k, m]
    wexpbv = (Wexp @ bv).astype(f32)

    def bf(x):
        return np.asarray(x, f32).astype(ml_dtypes.bfloat16).view(np.uint16)

    sm = np.zeros((P, SM_F), f32)
    u16 = sm.view(np.uint16).reshape(P, SM_F, 2)  # little-endian halves
    for t in range(NT):
        u16[:, SM_WSBF + t, 0] = bf(wS[t * P : (t + 1) * P])
        # wvet[t] is [k=128, m=256] -> 128 f32 cols of 2 bf16 each
        u16[:, SM_WVETBF + t * P : SM_WVETBF + (t + 1) * P, :] = bf(
            wvet[t * P : (t + 1) * P, :]
        ).reshape(P, P, 2)
        sm[:, SM_BEXPC + t] = bexp[t * P : (t + 1) * P]
    u16[0, SM_ONEBF, 0] = bf(1.0)
    u16[0, SM_WBVBF : SM_WBVBF + C // 2, :] = bf(wexpbv).reshape(C // 2, 2)
    sm[0, SM_BS] = bS
    return sm


def kernel(x, Wq, bq, Wk, bk, Wv, bv, Wcat, Wexp, bexp):
    global LAST_RESULTS
    f32 = np.float32
    x = np.ascontiguousarray(np.asarray(x, f32))
    args = [np.asarray(a, f32) for a in (Wq, bq, Wk, bk, Wv, bv, Wcat, Wexp, bexp)]
    sm = _pack_smalls(*args)

    if "prog" not in _prog_cache:
        _prog_cache["prog"] = _build()
    nc = _prog_cache["prog"]

    xf = x.reshape(B, C, N)
    in_maps = [
        {"xb": np.ascontiguousarray(xf[b]), "smalls": sm} for b in range(B)
    ]

    LAST_RESULTS = run_bass_kernel_spmd(nc, in_maps, core_ids=list(range(B)))
    out = np.stack([LAST_RESULTS.results[b]["out"] for b in range(B)], axis=0)
    return out.reshape(B, C, H, W).astype(f32)


if __name__ == "__main__":
    rng = np.random.default_rng(0)
    s = 0.02
    f32 = np.float32
    args = dict(
        x=rng.standard_normal((B, C, H, W)).astype(f32),
        Wq=(rng.standard_normal((E, C)) * s).astype(f32),
        bq=(rng.standard_normal((E,)) * s).astype(f32),
        Wk=(rng.standard_normal((E, C)) * s).astype(f32),
        bk=(rng.standard_normal((E,)) * s).astype(f32),
        Wv=(rng.standard_normal((E, C)) * s).astype(f32),
        bv=(rng.standard_normal((E,)) * s).astype(f32),
        Wcat=(rng.standard_normal((1, 2 * E)) * s).astype(f32),
        Wexp=(rng.standard_normal((C, E)) * s).astype(f32),
        bexp=(rng.standard_normal((C,)) * s).astype(f32),
    )
    o = kernel(**args)
    print(o.shape, o.dtype)



# revision 9
# speedup vs baseline: 1.0909x; 1.0909x over previous
"""ConcatNonLocalBlock kernel for 8x Trainium2 NeuronCores.

Math: the reference's attention matrix attn[b,i,j] = s[b,i]/n is constant
along j, so the whole block collapses to a rank-2 correction of x:

    out[b,c,i] = x[b,c,i] + bexp[c] + s[b,i] * uu[b,c]
    s[b,i]  = ReLU(wS . x[b,:,i] + bS)          wS = Wq^T wq_c + Wk^T wk_c
    uu[b,:] = (Wexp Wv) @ xsum[b] / n + Wexp bv  xsum[b,c] = sum_i x[b,c,i]

Sharding: data-parallel over batch, one sample per core (B=8, 8 cores).

v2: fully pipelined column-chunked dataflow. Input x streams in as 7
column chunks on the two HWDGE rings (sync + scalar, alternating so
per-DMA completion receipts overlap); per-chunk compute (bf16 casts,
row-sum partials, s matvec, ReLU) hides under the input stream. After
the last chunk a short uu chain runs, then pass 2 (outer product +
psum copy + x/bexp add + output DMA) pipelines against the output
stream.

Constraint shaping: this build rejects instructions with >1 sync wait,
so engine roles are chosen to merge semaphore waits:
  DVE    cast x tile0 -> bf16, ReLU (tensor_scalar), pass-2 adds (stt)
  ACT    cast x tile1 -> bf16, uu_bf copy, pass-2 psum->sbuf copies
  GpSimd row-sum partials + final reduce + xsum bf16 cast (no PSUM port)
  PE     s matvec, uu matmuls, outer products (+observer matmuls)
  Sync   smalls DMA + even in/out chunk DMAs; ACT issues odd ones
The matvec's psum WAR partner (ReLU, DVE) merges with its cast-t0 data
wait (DVE); the outer product's WAR partner (ACT copy) merges with its
uu_bf/s_bf data waits (ACT; s_bf covered by a PE observer matmul).
"""

import os
import sys

import numpy as np

sys.path.insert(0, "/opt/trn_rl_repo")

import concourse.bass as bass
import concourse.tile as tile
from concourse import mybir
from concourse.bass_utils import run_bass_kernel_spmd

B, C, H, W = 8, 256, 56, 56
N = H * W  # 3136
E = C // 2  # 128
P = 128  # SBUF partitions
NT = C // P  # 2 partition tiles
CW = 448  # column chunk width
CHUNKS = [(c0, CW) for c0 in range(0, N, CW)]  # 7 chunks
NCH = len(CHUNKS)
# 8 DMAs total (1 smalls + 4 in + 3 out) so each gets its own DMAHW
# semaphore lane -- lane recycling would add a second sync wait to the
# recycling DMA, which this build rejects.
ICHUNKS = [(0, 896), (896, 896), (1792, 896), (2688, 448)]
OCHUNKS = [(0, 1344), (1344, 896), (2240, 896)]

# packed "smalls" layout: [128, SM_F] f32 (bf16 payloads packed two-per-word,
# read on device via AP.bitcast(bfloat16))
SM_WSBF = 0  # cols 0..1 (bf16): col t halves = [ws[t*128+p], 0]
SM_WVETBF = 2  # cols 2..257 (bf16): t*128+j halves = Wve[2j+h, t*128+k]/N
SM_ONEBF = 258  # (bf16) [0, 258] low half = 1.0
SM_WBVBF = 259  # cols 259..386 (bf16): [0, 259+j] halves = (Wexp@bv)[2j+h]
SM_BS = 387  # f32 [0, 387] = bS (ReLU bias)
SM_BEXPC = 388  # f32 cols 388..389: col t = bexp[t*128+p] (per-partition scalar)
SM_F = 390

F32 = mybir.dt.float32

LAST_RESULTS = None  # BassKernelResults from the most recent run (for test.py)

_prog_cache = {}


class _SplitDrainTC(tile.TileContext):
    """TileContext whose exit drain splits its sem waits across single-wait
    NoOps: this walrus build rejects any instruction carrying more than one
    sync wait, and the stock kernel-tail drain collects the whole residual
    vector clock onto one Drain."""

    def _drain_and_barrier(self, tick_clock, wait_clock):
        from concourse.vector_clock import ScopedClock

        drain_inst = self.nc.sync.drain()
        wait_clock.add_sem_waits(
            drain_inst.ins, ScopedClock({None: tick_clock.global_clock})
        )
        si = drain_inst.ins.sync_info
        if si is not None and len(si.on_wait) > 1:
            waits = list(si.on_wait)
            drain_inst.ins.sync_info = mybir.SyncInfo(
                on_wait=[], on_update=list(si.on_update)
            )
            for w in waits:
                nop = self.nc.sync.nop()
                nop.ins.sync_info = mybir.SyncInfo(on_wait=[w], on_update=[])

        self.nc.all_engine_barrier()
        assert self.sems is not None
        popped = self.nc._tile_sem_poison_stack.pop()
        assert popped is self._sem_poison
        self.nc.clear_and_free_semaphores(list(self.sems.allocated().values()))
        self.nc.all_engine_barrier()


def _build():
    nc = bass.Bass()
    x_in = nc.dram_tensor("xb", [C, N], F32, kind="ExternalInput")
    sm_in = nc.dram_tensor("smalls", [P, SM_F], F32, kind="ExternalInput")
    out = nc.dram_tensor("out", [C, N], F32, kind="ExternalOutput")

    ADD = mybir.AluOpType.add
    MAX = mybir.AluOpType.max

    with _SplitDrainTC(nc) as tc:
        with (
            tc.tile_pool(name="persist", bufs=1) as persist,
            tc.tile_pool(name="ps_s", bufs=2, space="PSUM") as ps_s,
            tc.tile_pool(name="ps_u", bufs=1, space="PSUM") as ps_u,
            tc.tile_pool(name="ps_o", bufs=4, space="PSUM") as ps_o,
        ):
            BF16 = mybir.dt.bfloat16
            sm = persist.tile([P, SM_F], F32, tag="sm")
            nc.sync.dma_start(out=sm, in_=sm_in[:, :])

            def smbf(p0, p1, c0, c1):  # bf16 view of smalls cols [c0:c1)
                return sm[p0:p1, c0:c1].bitcast(BF16)

            xall = persist.tile([P, NT, N], F32, tag="xall")
            xbf = persist.tile([P, NT, N], BF16, tag="xbf")
            o_sb = persist.tile([P, NT, N], F32, tag="o_sb")
            s_bf = persist.tile([1, N], BF16, tag="s_bf")
            xs_part0 = persist.tile([P, NCH], F32, tag="xs_part0")
            xs_part1 = persist.tile([P, NCH], F32, tag="xs_part1")
            xs_part = [xs_part0, xs_part1]
            xsum_sb = persist.tile([P, NT], F32, tag="xsum_sb")
            xsum_bf = persist.tile([P, NT], BF16, tag="xsum_bf")
            uu_bf = persist.tile([1, C], BF16, tag="uu_bf")

            # input chunk DMAs: alternate the two HWDGE rings (sync /
            # scalar) so per-DMA completion receipts overlap instead of
            # serializing one ring; issues carry no waits
            for k, (c0, w) in enumerate(ICHUNKS):
                eng = nc.sync if k % 2 == 0 else nc.scalar
                eng.dma_start(
                    out=xall[:, :, c0 : c0 + w],
                    in_=x_in[:, c0 : c0 + w].rearrange("(t p) n -> p t n", p=P),
                )

            # Engine observers for the smalls DMA (so later ops carry only
            # their one data wait).
            dve_scratch = persist.tile([1, 1], F32, tag="dve_scratch")
            nc.vector.tensor_copy(out=dve_scratch, in_=sm[0:1, SM_BS : SM_BS + 1])
            act_scratch = persist.tile([1, 1], F32, tag="act_scratch")
            nc.scalar.copy(out=act_scratch, in_=sm[0:1, SM_BS : SM_BS + 1])
            uu_psum = ps_u.tile([1, C], F32, tag="uu_psum")
            nc.tensor.matmul(
                uu_psum[0:1, 0:1],
                lhsT=smbf(0, P, SM_WSBF, SM_WSBF + 1)[:, 0:1],
                rhs=smbf(0, P, SM_WSBF, SM_WSBF + 1)[:, 0:1],
                start=True,
                stop=True,
            )

            # ---- pass 1: per-chunk cast / rowsum / matvec / relu,
            # software-pipelined so the in-DMA-paced ops never sit behind
            # a cross-engine-dependent op on the same engine.
            s_ps = []

            def cast_chunk(ci):
                # cast f32->bf16 for the PE matvec; accum_out gives the
                # per-partition row-sum partial for free in the same op
                c0, w = CHUNKS[ci]
                nc.vector.tensor_scalar(
                    out=xbf[:, 0, c0 : c0 + w],
                    in0=xall[:, 0, c0 : c0 + w],
                    scalar1=0.0,
                    scalar2=0.0,
                    op0=ADD,
                    op1=ADD,
                    accum_out=xs_part[0][:, ci : ci + 1],
                )
                nc.vector.tensor_scalar(
                    out=xbf[:, 1, c0 : c0 + w],
                    in0=xall[:, 1, c0 : c0 + w],
                    scalar1=0.0,
                    scalar2=0.0,
                    op0=ADD,
                    op1=ADD,
                    accum_out=xs_part[1][:, ci : ci + 1],
                )

            def matvec_chunk(ci):
                c0, w = CHUNKS[ci]
                ps = ps_s.tile([1, CW], F32, tag="s_psum")
                s_ps.append(ps)
                for t in range(NT):
                    nc.tensor.matmul(
                        ps[:, :w],
                        lhsT=smbf(0, P, SM_WSBF + t, SM_WSBF + t + 1)[:, 0:1],
                        rhs=xbf[:, t, c0 : c0 + w],
                        start=(t == 0),
                        stop=(t == NT - 1),
                    )

            def relu_chunk(ci):
                c0, w = CHUNKS[ci]
                nc.vector.tensor_scalar(
                    out=s_bf[0:1, c0 : c0 + w],
                    in0=s_ps[ci][0:1, :w],
                    scalar1=sm[0:1, SM_BS : SM_BS + 1],
                    scalar2=0.0,
                    op0=ADD,
                    op1=MAX,
                )

            cast_chunk(0)
            matvec_chunk(0)
            for ci in range(1, NCH):
                cast_chunk(ci)
                matvec_chunk(ci)
                relu_chunk(ci - 1)
            relu_chunk(NCH - 1)

            # ---- uu chain (after last chunk's rowsum partials).  All on
            # DVE after relu(last) in program order, so the uu matmul's
            # single DVE wait transitively covers every s_bf write too.
            for t in range(NT):
                nc.vector.reduce_sum(
                    out=xsum_sb[:, t : t + 1],
                    in_=xs_part[t][:, :],
                    axis=mybir.AxisListType.X,
                )
            nc.vector.tensor_copy(out=xsum_bf[:, :], in_=xsum_sb[:, :])

            # uu = Wve/N @ xsum + Wexp bv  (row layout [1, C], bf16 matmuls)
            nc.tensor.matmul(
                uu_psum[:, :],
                lhsT=smbf(0, 1, SM_ONEBF, SM_ONEBF + 1)[:, 0:1],
                rhs=smbf(0, 1, SM_WBVBF, SM_WBVBF + C // 2),
                start=True,
                stop=False,
                skip_group_check=True,
            )
            for t in range(NT):
                nc.tensor.matmul(
                    uu_psum[:, :],
                    lhsT=xsum_bf[:, t : t + 1],
                    rhs=smbf(0, P, SM_WVETBF + t * P, SM_WVETBF + (t + 1) * P),
                    start=False,
                    stop=(t == NT - 1),
                    skip_group_check=True,
                )
            nc.scalar.copy(out=uu_bf[:, :], in_=uu_psum[:, :])

            # ---- pass 2: outer product (PE) -> psum copy (ACT) -> +x,+bexp
            # (DVE stt) -> chunk-pair output DMA (sync/scalar alternating)
            def outer_chunk(ci):
                # per-(chunk, tile) psum tiles, one matmul + one ACT copy
                # + one DVE stt each: keeps every hazard chain (WAW on the
                # psum bank, writer tracking on o_sb) exactly replaceable
                # so the framework's transitive wait elision kicks in
                c0, w = CHUNKS[ci]
                for t in range(NT):
                    po = ps_o.tile([P, CW], F32, tag="o_psum")
                    nc.tensor.matmul(
                        po[:, :w],
                        lhsT=uu_bf[0:1, t * P : (t + 1) * P],
                        rhs=s_bf[0:1, c0 : c0 + w],
                        start=True,
                        stop=True,
                    )
                    nc.scalar.copy(
                        out=o_sb[:, t, c0 : c0 + w], in_=po[:, :w]
                    )
                    nc.vector.scalar_tensor_tensor(
                        out=o_sb[:, t, c0 : c0 + w],
                        in0=o_sb[:, t, c0 : c0 + w],
                        scalar=sm[:, SM_BEXPC + t : SM_BEXPC + t + 1],
                        in1=xall[:, t, c0 : c0 + w],
                        op0=ADD,
                        op1=ADD,
                    )

            oc = 0
            for ci in range(NCH):
                outer_chunk(ci)
                while oc < len(OCHUNKS) and OCHUNKS[oc][0] + OCHUNKS[oc][1] <= (
                    CHUNKS[ci][0] + CHUNKS[ci][1]
                ):
                    c0, w = OCHUNKS[oc]
                    eng = nc.sync if oc % 2 == 0 else nc.scalar
                    eng.dma_start(
                        out=out[:, c0 : c0 + w].rearrange(
                            "(t p) n -> p t n", p=P
                        ),
                        in_=o_sb[:, :, c0 : c0 + w],
                    )
                    oc += 1
            assert oc == len(OCHUNKS)
    return nc


def _pack_smalls(Wq, bq, Wk, bk, Wv, bv, Wcat, Wexp, bexp):
    import ml_dtypes

    f32 = np.float32
    wq_c, wk_c = Wcat[0, :E], Wcat[0, E:]
    wS = (Wq.T @ wq_c + Wk.T @ wk_c).astype(f32)  # [C]
    bS = f32(wq_c @ bq + wk_c @ bk)
    Wve = (Wexp @ Wv).astype(f32)  # [C, C]
    wvet = (Wve.T / f32(N)).astype(f32)  # [C, C] : [k, m]
    wexpbv = (Wexp @ bv).astype(f32)

    def bf(x):
        return np.asarray(x, f32).astype(ml_dtypes.bfloat16).view(np.uint16)

    sm = np.zeros((P, SM_F), f32)
    u16 = sm.view(np.uint16).reshape(P, SM_F, 2)  # little-endian halves
    for t in range(NT):
        u16[:, SM_WSBF + t, 0] = bf(wS[t * P : (t + 1) * P])
        # wvet[t] is [k=128, m=256] -> 128 f32 cols of 2 bf16 each
        u16[:, SM_WVETBF + t * P : SM_WVETBF + (t + 1) * P, :] = bf(
            wvet[t * P : (t + 1) * P, :]
        ).reshape(P, P, 2)
        sm[:, SM_BEXPC + t] = bexp[t * P : (t + 1) * P]
    u16[0, SM_ONEBF, 0] = bf(1.0)
    u16[0, SM_WBVBF : SM_WBVBF + C // 2, :] = bf(wexpbv).reshape(C // 2, 2)
    sm[0, SM_BS] = bS
    return sm


def kernel(x, Wq, bq, Wk, bk, Wv, bv, Wcat, Wexp, bexp):
    global LAST_RESULTS
    f32 = np.float32
    x = np.ascontiguousarray(np.asarray(x, f32))
    args = [np.asarray(a, f32) for a in (Wq, bq, Wk, bk, Wv, bv, Wcat, Wexp, bexp)]
    sm = _pack_smalls(*args)

    if "prog" not in _prog_cache:
        _prog_cache["prog"] = _build()
    nc = _prog_cache["prog"]

    xf = x.reshape(B, C, N)
    in_maps = [
        {"xb": np.ascontiguousarray(xf[b]), "smalls": sm} for b in range(B)
    ]

    LAST_RESULTS = run_bass_kernel_spmd(nc, in_maps, core_ids=list(range(B)))
    out = np.stack([LAST_RESULTS.results[b]["out"] for b in range(B)], axis=0)
    return out.reshape(B, C, H, W).astype(f32)


if __name__ == "__main__":
    rng = np.random.default_rng(0)
    s = 0.02
    f32 = np.float32
    args = dict(
        x=rng.standard_normal((B, C, H, W)).astype(f32),
        Wq=(rng.standard_normal((E, C)) * s).astype(f32),
        bq=(rng.standard_normal((E,)) * s).astype(f32),
        Wk=(rng.standard_normal((E, C)) * s).astype(f32),
        bk=(rng.standard_normal((E,)) * s).astype(f32),
        Wv=(rng.standard_normal((E, C)) * s).astype(f32),
        bv=(rng.standard_normal((E,)) * s).astype(f32),
        Wcat=(rng.standard_normal((1, 2 * E)) * s).astype(f32),
        Wexp=(rng.standard_normal((C, E)) * s).astype(f32),
        bexp=(rng.standard_normal((C,)) * s).astype(f32),
    )
    o = kernel(**args)
    print(o.shape, o.dtype)


# revision 10
# speedup vs baseline: 1.1467x; 1.0512x over previous
"""ConcatNonLocalBlock kernel for 8x Trainium2 NeuronCores.

Math: the reference's attention matrix attn[b,i,j] = s[b,i]/n is constant
along j, so the whole block collapses to a rank-2 correction of x:

    out[b,c,i] = x[b,c,i] + bexp[c] + s[b,i] * uu[b,c]
    s[b,i]  = ReLU(wS . x[b,:,i] + bS)          wS = Wq^T wq_c + Wk^T wk_c
    uu[b,:] = (Wexp Wv) @ xsum[b] / n + Wexp bv  xsum[b,c] = sum_i x[b,c,i]

Sharding: data-parallel over batch, one sample per core (B=8, 8 cores).

v2: fully pipelined column-chunked dataflow. Input x streams in as 7
column chunks on the two HWDGE rings (sync + scalar, alternating so
per-DMA completion receipts overlap); per-chunk compute (bf16 casts,
row-sum partials, s matvec, ReLU) hides under the input stream. After
the last chunk a short uu chain runs, then pass 2 (outer product +
psum copy + x/bexp add + output DMA) pipelines against the output
stream.

Constraint shaping: this build rejects instructions with >1 sync wait,
so engine roles are chosen to merge semaphore waits:
  DVE    cast x tile0 -> bf16, ReLU (tensor_scalar), pass-2 adds (stt)
  ACT    cast x tile1 -> bf16, uu_bf copy, pass-2 psum->sbuf copies
  GpSimd row-sum partials + final reduce + xsum bf16 cast (no PSUM port)
  PE     s matvec, uu matmuls, outer products (+observer matmuls)
  Sync   smalls DMA + even in/out chunk DMAs; ACT issues odd ones
The matvec's psum WAR partner (ReLU, DVE) merges with its cast-t0 data
wait (DVE); the outer product's WAR partner (ACT copy) merges with its
uu_bf/s_bf data waits (ACT; s_bf covered by a PE observer matmul).
"""

import os
import sys

import numpy as np

sys.path.insert(0, "/opt/trn_rl_repo")

import concourse.bass as bass
import concourse.tile as tile
from concourse import mybir
from concourse.bass_utils import run_bass_kernel_spmd

B, C, H, W = 8, 256, 56, 56
N = H * W  # 3136
E = C // 2  # 128
P = 128  # SBUF partitions
NT = C // P  # 2 partition tiles
CW = 448  # column chunk width
CHUNKS = [(c0, CW) for c0 in range(0, N, CW)]  # 7 chunks
NCH = len(CHUNKS)
# 8 DMAs total (1 smalls + 4 in + 3 out) so each gets its own DMAHW
# semaphore lane -- lane recycling would add a second sync wait to the
# recycling DMA, which this build rejects.
ICHUNKS = [(0, 896), (896, 896), (1792, 896), (2688, 448)]
OCHUNKS = [(0, 1344), (1344, 896), (2240, 896)]

# packed "smalls" layout: [128, SM_F] f32 (bf16 payloads packed two-per-word,
# read on device via AP.bitcast(bfloat16))
SM_WSBF = 0  # cols 0..1 (bf16): col t halves = [ws[t*128+p], 0]
SM_WVETBF = 2  # cols 2..257 (bf16): t*128+j halves = Wve[2j+h, t*128+k]/N
SM_ONEBF = 258  # (bf16) [0, 258] low half = 1.0
SM_WBVBF = 259  # cols 259..386 (bf16): [0, 259+j] halves = (Wexp@bv)[2j+h]
SM_BS = 387  # f32 [0, 387] = bS (ReLU bias)
SM_BEXPC = 388  # f32 cols 388..389: col t = bexp[t*128+p] (per-partition scalar)
SM_F = 390

F32 = mybir.dt.float32

LAST_RESULTS = None  # BassKernelResults from the most recent run (for test.py)

_prog_cache = {}


class _SplitDrainTC(tile.TileContext):
    """TileContext whose exit drain splits its sem waits across single-wait
    NoOps: this walrus build rejects any instruction carrying more than one
    sync wait, and the stock kernel-tail drain collects the whole residual
    vector clock onto one Drain."""

    def _drain_and_barrier(self, tick_clock, wait_clock):
        from concourse.vector_clock import ScopedClock

        drain_inst = self.nc.sync.drain()
        wait_clock.add_sem_waits(
            drain_inst.ins, ScopedClock({None: tick_clock.global_clock})
        )
        si = drain_inst.ins.sync_info
        if si is not None and len(si.on_wait) > 1:
            waits = list(si.on_wait)
            drain_inst.ins.sync_info = mybir.SyncInfo(
                on_wait=[], on_update=list(si.on_update)
            )
            for w in waits:
                nop = self.nc.sync.nop()
                nop.ins.sync_info = mybir.SyncInfo(on_wait=[w], on_update=[])

        self.nc.all_engine_barrier()
        assert self.sems is not None
        popped = self.nc._tile_sem_poison_stack.pop()
        assert popped is self._sem_poison
        self.nc.clear_and_free_semaphores(list(self.sems.allocated().values()))
        self.nc.all_engine_barrier()


def _build():
    nc = bass.Bass()
    x_in = nc.dram_tensor("xb", [C, N], F32, kind="ExternalInput")
    sm_in = nc.dram_tensor("smalls", [P, SM_F], F32, kind="ExternalInput")
    out = nc.dram_tensor("out", [C, N], F32, kind="ExternalOutput")

    ADD = mybir.AluOpType.add
    MAX = mybir.AluOpType.max

    with _SplitDrainTC(nc) as tc:
        with (
            tc.tile_pool(name="persist", bufs=1) as persist,
            tc.tile_pool(name="ps_s", bufs=2, space="PSUM") as ps_s,
            tc.tile_pool(name="ps_u", bufs=1, space="PSUM") as ps_u,
            tc.tile_pool(name="ps_o", bufs=4, space="PSUM") as ps_o,
        ):
            BF16 = mybir.dt.bfloat16
            sm = persist.tile([P, SM_F], F32, tag="sm")
            nc.gpsimd.dma_start(out=sm, in_=sm_in[:, :])

            def smbf(p0, p1, c0, c1):  # bf16 view of smalls cols [c0:c1)
                return sm[p0:p1, c0:c1].bitcast(BF16)

            xall = persist.tile([P, NT, N], F32, tag="xall")
            xbf = persist.tile([P, NT, N], BF16, tag="xbf")
            o_sb = persist.tile([P, NT, N], F32, tag="o_sb")
            s_bf = persist.tile([1, N], BF16, tag="s_bf")
            xs_part0 = persist.tile([P, NCH], F32, tag="xs_part0")
            xs_part1 = persist.tile([P, NCH], F32, tag="xs_part1")
            xs_part = [xs_part0, xs_part1]
            xsum_sb = persist.tile([P, NT], F32, tag="xsum_sb")
            xsum_bf = persist.tile([P, NT], BF16, tag="xsum_bf")
            uu_bf = persist.tile([1, C], BF16, tag="uu_bf")

            # input chunk DMAs: alternate the two HWDGE rings (sync /
            # scalar) so per-DMA completion receipts overlap instead of
            # serializing one ring; issues carry no waits
            for k, (c0, w) in enumerate(ICHUNKS):
                eng = nc.sync if k % 2 == 0 else nc.scalar
                eng.dma_start(
                    out=xall[:, :, c0 : c0 + w],
                    in_=x_in[:, c0 : c0 + w].rearrange("(t p) n -> p t n", p=P),
                )

            # Engine observers for the smalls DMA (so later ops carry only
            # their one data wait).
            dve_scratch = persist.tile([1, 1], F32, tag="dve_scratch")
            nc.vector.tensor_copy(out=dve_scratch, in_=sm[0:1, SM_BS : SM_BS + 1])
            uu_psum = ps_u.tile([1, C], F32, tag="uu_psum")
            nc.tensor.matmul(
                uu_psum[0:1, 0:1],
                lhsT=smbf(0, P, SM_WSBF, SM_WSBF + 1)[:, 0:1],
                rhs=smbf(0, P, SM_WSBF, SM_WSBF + 1)[:, 0:1],
                start=True,
                stop=True,
            )

            # ---- pass 1: per-chunk cast / rowsum / matvec / relu,
            # software-pipelined so the in-DMA-paced ops never sit behind
            # a cross-engine-dependent op on the same engine.
            s_ps = []

            def cast_chunk(ci):
                # cast f32->bf16 for the PE matvec; accum_out gives the
                # per-partition row-sum partial for free in the same op
                c0, w = CHUNKS[ci]
                nc.vector.tensor_scalar(
                    out=xbf[:, 0, c0 : c0 + w],
                    in0=xall[:, 0, c0 : c0 + w],
                    scalar1=0.0,
                    scalar2=0.0,
                    op0=ADD,
                    op1=ADD,
                    accum_out=xs_part[0][:, ci : ci + 1],
                )
                nc.scalar.activation(
                    out=xbf[:, 1, c0 : c0 + w],
                    in_=xall[:, 1, c0 : c0 + w],
                    func=mybir.ActivationFunctionType.Copy,
                    accum_out=xs_part[1][:, ci : ci + 1],
                )

            def matvec_chunk(ci):
                c0, w = CHUNKS[ci]
                ps = ps_s.tile([1, CW], F32, tag="s_psum")
                s_ps.append(ps)
                for t in range(NT):
                    nc.tensor.matmul(
                        ps[:, :w],
                        lhsT=smbf(0, P, SM_WSBF + t, SM_WSBF + t + 1)[:, 0:1],
                        rhs=xbf[:, t, c0 : c0 + w],
                        start=(t == 0),
                        stop=(t == NT - 1),
                    )

            def relu_chunk(ci):
                c0, w = CHUNKS[ci]
                nc.vector.tensor_scalar(
                    out=s_bf[0:1, c0 : c0 + w],
                    in0=s_ps[ci][0:1, :w],
                    scalar1=sm[0:1, SM_BS : SM_BS + 1],
                    scalar2=0.0,
                    op0=ADD,
                    op1=MAX,
                )

            cast_chunk(0)
            matvec_chunk(0)
            for ci in range(1, NCH):
                cast_chunk(ci)
                matvec_chunk(ci)
                relu_chunk(ci - 1)
            relu_chunk(NCH - 1)

            # ---- uu chain (after last chunk's rowsum partials).  All on
            # DVE after relu(last) in program order, so the uu matmul's
            # single DVE wait transitively covers every s_bf write too.
            for t in range(NT):
                nc.vector.reduce_sum(
                    out=xsum_sb[:, t : t + 1],
                    in_=xs_part[t][:, :],
                    axis=mybir.AxisListType.X,
                )
            nc.vector.tensor_copy(out=xsum_bf[:, :], in_=xsum_sb[:, :])

            # uu = Wve/N @ xsum + Wexp bv  (row layout [1, C], bf16 matmuls)
            nc.tensor.matmul(
                uu_psum[:, :],
                lhsT=smbf(0, 1, SM_ONEBF, SM_ONEBF + 1)[:, 0:1],
                rhs=smbf(0, 1, SM_WBVBF, SM_WBVBF + C // 2),
                start=True,
                stop=False,
                skip_group_check=True,
            )
            for t in range(NT):
                nc.tensor.matmul(
                    uu_psum[:, :],
                    lhsT=xsum_bf[:, t : t + 1],
                    rhs=smbf(0, P, SM_WVETBF + t * P, SM_WVETBF + (t + 1) * P),
                    start=False,
                    stop=(t == NT - 1),
                    skip_group_check=True,
                )
            nc.vector.tensor_copy(out=uu_bf[:, :], in_=uu_psum[:, :])

            # ---- pass 2: outer product (PE) -> psum copy (ACT) -> +x,+bexp
            # (DVE stt) -> chunk-pair output DMA (sync/scalar alternating)
            def outer_chunk(ci):
                # per-(chunk, tile) psum tiles, one matmul + one ACT copy
                # + one DVE stt each: keeps every hazard chain (WAW on the
                # psum bank, writer tracking on o_sb) exactly replaceable
                # so the framework's transitive wait elision kicks in
                c0, w = CHUNKS[ci]
                for t in range(NT):
                    po = ps_o.tile([P, CW], F32, tag="o_psum")
                    nc.tensor.matmul(
                        po[:, :w],
                        lhsT=uu_bf[0:1, t * P : (t + 1) * P],
                        rhs=s_bf[0:1, c0 : c0 + w],
                        start=True,
                        stop=True,
                    )
                    nc.vector.scalar_tensor_tensor(
                        out=o_sb[:, t, c0 : c0 + w],
                        in0=xall[:, t, c0 : c0 + w],
                        scalar=sm[:, SM_BEXPC + t : SM_BEXPC + t + 1],
                        in1=po[:, :w],
                        op0=ADD,
                        op1=ADD,
                    )

            oc = 0
            for ci in range(NCH):
                outer_chunk(ci)
                while oc < len(OCHUNKS) and OCHUNKS[oc][0] + OCHUNKS[oc][1] <= (
                    CHUNKS[ci][0] + CHUNKS[ci][1]
                ):
                    c0, w = OCHUNKS[oc]
                    eng = nc.sync if oc % 2 == 0 else nc.scalar
                    eng.dma_start(
                        out=out[:, c0 : c0 + w].rearrange(
                            "(t p) n -> p t n", p=P
                        ),
                        in_=o_sb[:, :, c0 : c0 + w],
                    )
                    oc += 1
            assert oc == len(OCHUNKS)
    return nc


def _pack_smalls(Wq, bq, Wk, bk, Wv, bv, Wcat, Wexp, bexp):
    import ml_dtypes

    f32 = np.float32
    wq_c, wk_c = Wcat[0, :E], Wcat[0, E:]
    wS = (Wq.T @ wq_c + Wk.T @ wk_c).astype(f32)  # [C]
    bS = f32(wq_c @ bq + wk_c @ bk)
    Wve = (Wexp @ Wv).astype(f32)  # [C, C]
    wvet = (Wve.T / f32(N)).astype(f32)  # [C, C] : [k, m]
    wexpbv = (Wexp @ bv).astype(f32)

    def bf(x):
        return np.asarray(x, f32).astype(ml_dtypes.bfloat16).view(np.uint16)

    sm = np.zeros((P, SM_F), f32)
    u16 = sm.view(np.uint16).reshape(P, SM_F, 2)  # little-endian halves
    for t in range(NT):
        u16[:, SM_WSBF + t, 0] = bf(wS[t * P : (t + 1) * P])
        # wvet[t] is [k=128, m=256] -> 128 f32 cols of 2 bf16 each
        u16[:, SM_WVETBF + t * P : SM_WVETBF + (t + 1) * P, :] = bf(
            wvet[t * P : (t + 1) * P, :]
        ).reshape(P, P, 2)
        sm[:, SM_BEXPC + t] = bexp[t * P : (t + 1) * P]
    u16[0, SM_ONEBF, 0] = bf(1.0)
    u16[0, SM_WBVBF : SM_WBVBF + C // 2, :] = bf(wexpbv).reshape(C // 2, 2)
    sm[0, SM_BS] = bS
    return sm


def kernel(x, Wq, bq, Wk, bk, Wv, bv, Wcat, Wexp, bexp):
    global LAST_RESULTS
    f32 = np.float32
    x = np.ascontiguousarray(np.asarray(x, f32))
    args = [np.asarray(a, f32) for a in (Wq, bq, Wk, bk, Wv, bv, Wcat, Wexp, bexp)]
    sm = _pack_smalls(*args)

    if "prog" not in _prog_cache:
        _prog_cache["prog"] = _build()
    nc = _prog_cache["prog"]

    xf = x.reshape(B, C, N)
    in_maps = [
        {"xb": np.ascontiguousarray(xf[b]), "smalls": sm} for b in range(B)
    ]

    LAST_RESULTS = run_bass_kernel_spmd(nc, in_maps, core_ids=list(range(B)))
    out = np.stack([LAST_RESULTS.results[b]["out"] for b in range(B)], axis=0)
    return out.reshape(B, C, H, W).astype(f32)


if __name__ == "__main__":
    rng = np.random.default_rng(0)
    s = 0.02
    f32 = np.float32
    args = dict(
        x=rng.standard_normal((B, C, H, W)).astype(f32),
        Wq=(rng.standard_normal((E, C)) * s).astype(f32),
        bq=(rng.standard_normal((E,)) * s).astype(f32),
        Wk=(rng.standard_normal((E, C)) * s).astype(f32),
        bk=(rng.standard_normal((E,)) * s).astype(f32),
        Wv=(rng.standard_normal((E, C)) * s).astype(f32),
        bv=(rng.standard_normal((E,)) * s).astype(f32),
        Wcat=(rng.standard_normal((1, 2 * E)) * s).astype(f32),
        Wexp=(rng.standard_normal((C, E)) * s).astype(f32),
        bexp=(rng.standard_normal((C,)) * s).astype(f32),
    )
    o = kernel(**args)
    print(o.shape, o.dtype)


# revision 12
# speedup vs baseline: 1.2163x; 1.0606x over previous
"""ConcatNonLocalBlock kernel for 8x Trainium2 NeuronCores.

Math: the reference's attention matrix attn[b,i,j] = s[b,i]/n is constant
along j, so the whole block collapses to a rank-2 correction of x:

    out[b,c,i] = x[b,c,i] + bexp[c] + s[b,i] * uu[b,c]
    s[b,i]  = ReLU(wS . x[b,:,i] + bS)          wS = Wq^T wq_c + Wk^T wk_c
    uu[b,:] = (Wexp Wv) @ xsum[b] / n + Wexp bv  xsum[b,c] = sum_i x[b,c,i]

Sharding: data-parallel over batch, one sample per core (B=8, 8 cores).

v2: fully pipelined column-chunked dataflow. Input x streams in as 7
column chunks on the two HWDGE rings (sync + scalar, alternating so
per-DMA completion receipts overlap); per-chunk compute (bf16 casts,
row-sum partials, s matvec, ReLU) hides under the input stream. After
the last chunk a short uu chain runs, then pass 2 (outer product +
psum copy + x/bexp add + output DMA) pipelines against the output
stream.

Constraint shaping: this build rejects instructions with >1 sync wait,
so engine roles are chosen to merge semaphore waits:
  DVE    cast x tile0 -> bf16, ReLU (tensor_scalar), pass-2 adds (stt)
  ACT    cast x tile1 -> bf16, uu_bf copy, pass-2 psum->sbuf copies
  GpSimd row-sum partials + final reduce + xsum bf16 cast (no PSUM port)
  PE     s matvec, uu matmuls, outer products (+observer matmuls)
  Sync   smalls DMA + even in/out chunk DMAs; ACT issues odd ones
The matvec's psum WAR partner (ReLU, DVE) merges with its cast-t0 data
wait (DVE); the outer product's WAR partner (ACT copy) merges with its
uu_bf/s_bf data waits (ACT; s_bf covered by a PE observer matmul).
"""

import os
import sys

import numpy as np

sys.path.insert(0, "/opt/trn_rl_repo")

import concourse.bass as bass
import concourse.tile as tile
from concourse import mybir
from concourse.bass_utils import run_bass_kernel_spmd

B, C, H, W = 8, 256, 56, 56
N = H * W  # 3136
E = C // 2  # 128
P = 128  # SBUF partitions
NT = C // P  # 2 partition tiles
CW = 448  # column chunk width
CHUNKS = [(c0, CW) for c0 in range(0, N, CW)]  # 7 chunks
NCH = len(CHUNKS)
# 8 DMAs total (1 smalls + 4 in + 3 out) so each gets its own DMAHW
# semaphore lane -- lane recycling would add a second sync wait to the
# recycling DMA, which this build rejects.
ICHUNKS = [(0, 896), (896, 896), (1792, 1344)]
OCHUNKS = [(0, 896), (896, 896), (1792, 896), (2688, 448)]

# packed "smalls" layout: [128, SM_F] f32 (bf16 payloads packed two-per-word,
# read on device via AP.bitcast(bfloat16))
SM_WSBF = 0  # cols 0..1 (bf16): col t halves = [ws[t*128+p], 0]
SM_WVETBF = 2  # cols 2..257 (bf16): t*128+j halves = Wve[2j+h, t*128+k]/N
SM_ONEBF = 258  # (bf16) [0, 258] low half = 1.0
SM_WBVBF = 259  # cols 259..386 (bf16): [0, 259+j] halves = (Wexp@bv)[2j+h]
SM_BS = 387  # f32 [0, 387] = bS (ReLU bias)
SM_BEXPC = 388  # f32 cols 388..389: col t = bexp[t*128+p] (per-partition scalar)
SM_F = 390

F32 = mybir.dt.float32

LAST_RESULTS = None  # BassKernelResults from the most recent run (for test.py)

_prog_cache = {}


class _SplitDrainTC(tile.TileContext):
    """TileContext whose exit drain splits its sem waits across single-wait
    NoOps: this walrus build rejects any instruction carrying more than one
    sync wait, and the stock kernel-tail drain collects the whole residual
    vector clock onto one Drain."""

    def _drain_and_barrier(self, tick_clock, wait_clock):
        from concourse.vector_clock import ScopedClock

        drain_inst = self.nc.sync.drain()
        wait_clock.add_sem_waits(
            drain_inst.ins, ScopedClock({None: tick_clock.global_clock})
        )
        si = drain_inst.ins.sync_info
        if si is not None and len(si.on_wait) > 1:
            waits = list(si.on_wait)
            drain_inst.ins.sync_info = mybir.SyncInfo(
                on_wait=[], on_update=list(si.on_update)
            )
            for w in waits:
                nop = self.nc.sync.nop()
                nop.ins.sync_info = mybir.SyncInfo(on_wait=[w], on_update=[])

        self.nc.all_engine_barrier()
        assert self.sems is not None
        popped = self.nc._tile_sem_poison_stack.pop()
        assert popped is self._sem_poison
        self.nc.clear_and_free_semaphores(list(self.sems.allocated().values()))
        self.nc.all_engine_barrier()


def _build():
    nc = bass.Bass()
    x_in = nc.dram_tensor("xb", [C, N], F32, kind="ExternalInput")
    sm_in = nc.dram_tensor("smalls", [P, SM_F], F32, kind="ExternalInput")
    out = nc.dram_tensor("out", [C, N], F32, kind="ExternalOutput")

    ADD = mybir.AluOpType.add
    MAX = mybir.AluOpType.max

    with _SplitDrainTC(nc) as tc:
        with (
            tc.tile_pool(name="persist", bufs=1) as persist,
            tc.tile_pool(name="ps_s", bufs=2, space="PSUM") as ps_s,
            tc.tile_pool(name="ps_u", bufs=1, space="PSUM") as ps_u,
            tc.tile_pool(name="ps_o", bufs=4, space="PSUM") as ps_o,
        ):
            BF16 = mybir.dt.bfloat16
            sm = persist.tile([P, SM_F], F32, tag="sm")
            nc.scalar.dma_start(out=sm, in_=sm_in[:, :])

            def smbf(p0, p1, c0, c1):  # bf16 view of smalls cols [c0:c1)
                return sm[p0:p1, c0:c1].bitcast(BF16)

            xall = persist.tile([P, NT, N], F32, tag="xall")
            xbf = persist.tile([P, NT, N], BF16, tag="xbf")
            o_sb = persist.tile([P, NT, N], F32, tag="o_sb")
            s_bf = persist.tile([1, N], BF16, tag="s_bf")
            xs_part0 = persist.tile([P, NCH], F32, tag="xs_part0")
            xs_part1 = persist.tile([P, NCH], F32, tag="xs_part1")
            xs_part = [xs_part0, xs_part1]
            xsum_sb = persist.tile([P, NT], F32, tag="xsum_sb")
            xsum_bf = persist.tile([P, NT], BF16, tag="xsum_bf")
            uu_bf = persist.tile([1, C], BF16, tag="uu_bf")

            for k, (c0, w) in enumerate(ICHUNKS):
                nc.sync.dma_start(
                    out=xall[:, :, c0 : c0 + w],
                    in_=x_in[:, c0 : c0 + w].rearrange("(t p) n -> p t n", p=P),
                )

            # Engine observers for the smalls DMA (so later ops carry only
            # their one data wait).
            dve_scratch = persist.tile([1, 1], F32, tag="dve_scratch")
            nc.vector.tensor_copy(out=dve_scratch, in_=sm[0:1, SM_BS : SM_BS + 1])
            uu_psum = ps_u.tile([1, C], F32, tag="uu_psum")
            nc.tensor.matmul(
                uu_psum[0:1, 0:1],
                lhsT=smbf(0, P, SM_WSBF, SM_WSBF + 1)[:, 0:1],
                rhs=smbf(0, P, SM_WSBF, SM_WSBF + 1)[:, 0:1],
                start=True,
                stop=True,
            )

            # ---- pass 1: per-chunk cast / rowsum / matvec / relu,
            # software-pipelined so the in-DMA-paced ops never sit behind
            # a cross-engine-dependent op on the same engine.
            s_ps = []

            def cast_chunk(ci):
                # cast f32->bf16 for the PE matvec; accum_out gives the
                # per-partition row-sum partial for free in the same op
                c0, w = CHUNKS[ci]
                nc.vector.tensor_scalar(
                    out=xbf[:, 0, c0 : c0 + w],
                    in0=xall[:, 0, c0 : c0 + w],
                    scalar1=0.0,
                    scalar2=0.0,
                    op0=ADD,
                    op1=ADD,
                    accum_out=xs_part[0][:, ci : ci + 1],
                )
                nc.scalar.activation(
                    out=xbf[:, 1, c0 : c0 + w],
                    in_=xall[:, 1, c0 : c0 + w],
                    func=mybir.ActivationFunctionType.Copy,
                    accum_out=xs_part[1][:, ci : ci + 1],
                )

            def matvec_chunk(ci):
                c0, w = CHUNKS[ci]
                ps = ps_s.tile([1, CW], F32, tag="s_psum")
                s_ps.append(ps)
                for t in range(NT):
                    nc.tensor.matmul(
                        ps[:, :w],
                        lhsT=smbf(0, P, SM_WSBF + t, SM_WSBF + t + 1)[:, 0:1],
                        rhs=xbf[:, t, c0 : c0 + w],
                        start=(t == 0),
                        stop=(t == NT - 1),
                    )

            def relu_chunk(ci):
                c0, w = CHUNKS[ci]
                nc.vector.tensor_scalar(
                    out=s_bf[0:1, c0 : c0 + w],
                    in0=s_ps[ci][0:1, :w],
                    scalar1=sm[0:1, SM_BS : SM_BS + 1],
                    scalar2=0.0,
                    op0=ADD,
                    op1=MAX,
                )

            cast_chunk(0)
            matvec_chunk(0)
            for ci in range(1, NCH):
                cast_chunk(ci)
                matvec_chunk(ci)
                relu_chunk(ci - 1)
            relu_chunk(NCH - 1)

            # ---- uu chain (after last chunk's rowsum partials).  All on
            # DVE after relu(last) in program order, so the uu matmul's
            # single DVE wait transitively covers every s_bf write too.
            for t in range(NT):
                nc.vector.reduce_sum(
                    out=xsum_sb[:, t : t + 1],
                    in_=xs_part[t][:, :],
                    axis=mybir.AxisListType.X,
                )
            nc.vector.tensor_copy(out=xsum_bf[:, :], in_=xsum_sb[:, :])

            # uu = Wve/N @ xsum + Wexp bv  (row layout [1, C], bf16 matmuls)
            nc.tensor.matmul(
                uu_psum[:, :],
                lhsT=smbf(0, 1, SM_ONEBF, SM_ONEBF + 1)[:, 0:1],
                rhs=smbf(0, 1, SM_WBVBF, SM_WBVBF + C // 2),
                start=True,
                stop=False,
                skip_group_check=True,
            )
            for t in range(NT):
                nc.tensor.matmul(
                    uu_psum[:, :],
                    lhsT=xsum_bf[:, t : t + 1],
                    rhs=smbf(0, P, SM_WVETBF + t * P, SM_WVETBF + (t + 1) * P),
                    start=False,
                    stop=(t == NT - 1),
                    skip_group_check=True,
                )
            nc.vector.tensor_copy(out=uu_bf[:, :], in_=uu_psum[:, :])

            # ---- pass 2: outer product (PE) -> psum copy (ACT) -> +x,+bexp
            # (DVE stt) -> chunk-pair output DMA (sync/scalar alternating)
            def outer_chunk(ci):
                # per-(chunk, tile) psum tiles, one matmul + one ACT copy
                # + one DVE stt each: keeps every hazard chain (WAW on the
                # psum bank, writer tracking on o_sb) exactly replaceable
                # so the framework's transitive wait elision kicks in
                c0, w = CHUNKS[ci]
                for t in range(NT):
                    po = ps_o.tile([P, CW], F32, tag="o_psum")
                    nc.tensor.matmul(
                        po[:, :w],
                        lhsT=uu_bf[0:1, t * P : (t + 1) * P],
                        rhs=s_bf[0:1, c0 : c0 + w],
                        start=True,
                        stop=True,
                    )
                    nc.vector.scalar_tensor_tensor(
                        out=o_sb[:, t, c0 : c0 + w],
                        in0=xall[:, t, c0 : c0 + w],
                        scalar=sm[:, SM_BEXPC + t : SM_BEXPC + t + 1],
                        in1=po[:, :w],
                        op0=ADD,
                        op1=ADD,
                    )

            oc = 0
            for ci in range(NCH):
                outer_chunk(ci)
                while oc < len(OCHUNKS) and OCHUNKS[oc][0] + OCHUNKS[oc][1] <= (
                    CHUNKS[ci][0] + CHUNKS[ci][1]
                ):
                    c0, w = OCHUNKS[oc]
                    eng = nc.sync if oc % 2 == 0 else nc.scalar
                    eng.dma_start(
                        out=out[:, c0 : c0 + w].rearrange(
                            "(t p) n -> p t n", p=P
                        ),
                        in_=o_sb[:, :, c0 : c0 + w],
                    )
                    oc += 1
            assert oc == len(OCHUNKS)
    return nc


def _pack_smalls(Wq, bq, Wk, bk, Wv, bv, Wcat, Wexp, bexp):
    import ml_dtypes

    f32 = np.float32
    wq_c, wk_c = Wcat[0, :E], Wcat[0, E:]
    wS = (Wq.T @ wq_c + Wk.T @ wk_c).astype(f32)  # [C]
    bS = f32(wq_c @ bq + wk_c @ bk)
    Wve = (Wexp @ Wv).astype(f32)  # [C, C]
    wvet = (Wve.T / f32(N)).astype(f32)  # [C, C] : [k, m]
    wexpbv = (Wexp @ bv).astype(f32)

    def bf(x):
        return np.asarray(x, f32).astype(ml_dtypes.bfloat16).view(np.uint16)

    sm = np.zeros((P, SM_F), f32)
    u16 = sm.view(np.uint16).reshape(P, SM_F, 2)  # little-endian halves
    for t in range(NT):
        u16[:, SM_WSBF + t, 0] = bf(wS[t * P : (t + 1) * P])
        # wvet[t] is [k=128, m=256] -> 128 f32 cols of 2 bf16 each
        u16[:, SM_WVETBF + t * P : SM_WVETBF + (t + 1) * P, :] = bf(
            wvet[t * P : (t + 1) * P, :]
        ).reshape(P, P, 2)
        sm[:, SM_BEXPC + t] = bexp[t * P : (t + 1) * P]
    u16[0, SM_ONEBF, 0] = bf(1.0)
    u16[0, SM_WBVBF : SM_WBVBF + C // 2, :] = bf(wexpbv).reshape(C // 2, 2)
    sm[0, SM_BS] = bS
    return sm


def kernel(x, Wq, bq, Wk, bk, Wv, bv, Wcat, Wexp, bexp):
    global LAST_RESULTS
    f32 = np.float32
    x = np.ascontiguousarray(np.asarray(x, f32))
    args = [np.asarray(a, f32) for a in (Wq, bq, Wk, bk, Wv, bv, Wcat, Wexp, bexp)]
    sm = _pack_smalls(*args)

    if "prog" not in _prog_cache:
        _prog_cache["prog"] = _build()
    nc = _prog_cache["prog"]

    xf = x.reshape(B, C, N)
    in_maps = [
        {"xb": np.ascontiguousarray(xf[b]), "smalls": sm} for b in range(B)
    ]

    LAST_RESULTS = run_bass_kernel_spmd(nc, in_maps, core_ids=list(range(B)))
    out = np.stack([LAST_RESULTS.results[b]["out"] for b in range(B)], axis=0)
    return out.reshape(B, C, H, W).astype(f32)


if __name__ == "__main__":
    rng = np.random.default_rng(0)
    s = 0.02
    f32 = np.float32
    args = dict(
        x=rng.standard_normal((B, C, H, W)).astype(f32),
        Wq=(rng.standard_normal((E, C)) * s).astype(f32),
        bq=(rng.standard_normal((E,)) * s).astype(f32),
        Wk=(rng.standard_normal((E, C)) * s).astype(f32),
        bk=(rng.standard_normal((E,)) * s).astype(f32),
        Wv=(rng.standard_normal((E, C)) * s).astype(f32),
        bv=(rng.standard_normal((E,)) * s).astype(f32),
        Wcat=(rng.standard_normal((1, 2 * E)) * s).astype(f32),
        Wexp=(rng.standard_normal((C, E)) * s).astype(f32),
        bexp=(rng.standard_normal((C,)) * s).astype(f32),
    )
    o = kernel(**args)
    print(o.shape, o.dtype)


# revision 13
# speedup vs baseline: 1.2389x; 1.0186x over previous
"""ConcatNonLocalBlock kernel for 8x Trainium2 NeuronCores.

Math: the reference's attention matrix attn[b,i,j] = s[b,i]/n is constant
along j, so the whole block collapses to a rank-2 correction of x:

    out[b,c,i] = x[b,c,i] + bexp[c] + s[b,i] * uu[b,c]
    s[b,i]  = ReLU(wS . x[b,:,i] + bS)          wS = Wq^T wq_c + Wk^T wk_c
    uu[b,:] = (Wexp Wv) @ xsum[b] / n + Wexp bv  xsum[b,c] = sum_i x[b,c,i]

Sharding: data-parallel over batch, one sample per core (B=8, 8 cores).

v2: fully pipelined column-chunked dataflow. Input x streams in as 7
column chunks on the two HWDGE rings (sync + scalar, alternating so
per-DMA completion receipts overlap); per-chunk compute (bf16 casts,
row-sum partials, s matvec, ReLU) hides under the input stream. After
the last chunk a short uu chain runs, then pass 2 (outer product +
psum copy + x/bexp add + output DMA) pipelines against the output
stream.

Constraint shaping: this build rejects instructions with >1 sync wait,
so engine roles are chosen to merge semaphore waits:
  DVE    cast x tile0 -> bf16, ReLU (tensor_scalar), pass-2 adds (stt)
  ACT    cast x tile1 -> bf16, uu_bf copy, pass-2 psum->sbuf copies
  GpSimd row-sum partials + final reduce + xsum bf16 cast (no PSUM port)
  PE     s matvec, uu matmuls, outer products (+observer matmuls)
  Sync   smalls DMA + even in/out chunk DMAs; ACT issues odd ones
The matvec's psum WAR partner (ReLU, DVE) merges with its cast-t0 data
wait (DVE); the outer product's WAR partner (ACT copy) merges with its
uu_bf/s_bf data waits (ACT; s_bf covered by a PE observer matmul).
"""

import os
import sys

import numpy as np

sys.path.insert(0, "/opt/trn_rl_repo")

import concourse.bass as bass
import concourse.tile as tile
from concourse import mybir
from concourse.bass_utils import run_bass_kernel_spmd

B, C, H, W = 8, 256, 56, 56
N = H * W  # 3136
E = C // 2  # 128
P = 128  # SBUF partitions
NT = C // P  # 2 partition tiles
CW = 448  # column chunk width
CHUNKS = [(c0, CW) for c0 in range(0, N, CW)]  # 7 chunks
NCH = len(CHUNKS)
# 8 DMAs total (1 smalls + 4 in + 3 out) so each gets its own DMAHW
# semaphore lane -- lane recycling would add a second sync wait to the
# recycling DMA, which this build rejects.
ICHUNKS = [(0, 1344), (1344, 1344), (2688, 448)]
OCHUNKS = [(0, 896), (896, 896), (1792, 896), (2688, 448)]

# packed "smalls" layout: [128, SM_F] f32 (bf16 payloads packed two-per-word,
# read on device via AP.bitcast(bfloat16))
SM_WSBF = 0  # cols 0..1 (bf16): col t halves = [ws[t*128+p], 0]
SM_WVETBF = 2  # cols 2..257 (bf16): t*128+j halves = Wve[2j+h, t*128+k]/N
SM_ONEBF = 258  # (bf16) [0, 258] low half = 1.0
SM_WBVBF = 259  # cols 259..386 (bf16): [0, 259+j] halves = (Wexp@bv)[2j+h]
SM_BS = 387  # f32 [0, 387] = bS (ReLU bias)
SM_BEXPC = 388  # f32 cols 388..389: col t = bexp[t*128+p] (per-partition scalar)
SM_F = 390

F32 = mybir.dt.float32

LAST_RESULTS = None  # BassKernelResults from the most recent run (for test.py)

_prog_cache = {}


class _SplitDrainTC(tile.TileContext):
    """TileContext whose exit drain splits its sem waits across single-wait
    NoOps: this walrus build rejects any instruction carrying more than one
    sync wait, and the stock kernel-tail drain collects the whole residual
    vector clock onto one Drain."""

    def _drain_and_barrier(self, tick_clock, wait_clock):
        from concourse.vector_clock import ScopedClock

        drain_inst = self.nc.sync.drain()
        wait_clock.add_sem_waits(
            drain_inst.ins, ScopedClock({None: tick_clock.global_clock})
        )
        si = drain_inst.ins.sync_info
        if si is not None and len(si.on_wait) > 1:
            waits = list(si.on_wait)
            drain_inst.ins.sync_info = mybir.SyncInfo(
                on_wait=[], on_update=list(si.on_update)
            )
            for w in waits:
                nop = self.nc.sync.nop()
                nop.ins.sync_info = mybir.SyncInfo(on_wait=[w], on_update=[])

        self.nc.all_engine_barrier()
        assert self.sems is not None
        popped = self.nc._tile_sem_poison_stack.pop()
        assert popped is self._sem_poison
        self.nc.clear_and_free_semaphores(list(self.sems.allocated().values()))
        self.nc.all_engine_barrier()


def _build():
    nc = bass.Bass()
    x_in = nc.dram_tensor("xb", [C, N], F32, kind="ExternalInput")
    sm_in = nc.dram_tensor("smalls", [P, SM_F], F32, kind="ExternalInput")
    out = nc.dram_tensor("out", [C, N], F32, kind="ExternalOutput")

    ADD = mybir.AluOpType.add
    MAX = mybir.AluOpType.max

    with _SplitDrainTC(nc) as tc:
        with (
            tc.tile_pool(name="persist", bufs=1) as persist,
            tc.tile_pool(name="ps_s", bufs=2, space="PSUM") as ps_s,
            tc.tile_pool(name="ps_u", bufs=1, space="PSUM") as ps_u,
            tc.tile_pool(name="ps_o", bufs=4, space="PSUM") as ps_o,
        ):
            BF16 = mybir.dt.bfloat16
            sm = persist.tile([P, SM_F], F32, tag="sm")
            nc.scalar.dma_start(out=sm, in_=sm_in[:, :])

            def smbf(p0, p1, c0, c1):  # bf16 view of smalls cols [c0:c1)
                return sm[p0:p1, c0:c1].bitcast(BF16)

            xall = persist.tile([P, NT, N], F32, tag="xall")
            xbf = persist.tile([P, NT, N], BF16, tag="xbf")
            o_sb = persist.tile([P, NT, N], F32, tag="o_sb")
            s_bf = persist.tile([1, N], BF16, tag="s_bf")
            xs_part0 = persist.tile([P, NCH], F32, tag="xs_part0")
            xs_part1 = persist.tile([P, NCH], F32, tag="xs_part1")
            xs_part = [xs_part0, xs_part1]
            xsum_sb = persist.tile([P, NT], F32, tag="xsum_sb")
            xsum_bf = persist.tile([P, NT], BF16, tag="xsum_bf")
            uu_bf = persist.tile([1, C], BF16, tag="uu_bf")

            for k, (c0, w) in enumerate(ICHUNKS):
                nc.sync.dma_start(
                    out=xall[:, :, c0 : c0 + w],
                    in_=x_in[:, c0 : c0 + w].rearrange("(t p) n -> p t n", p=P),
                )

            # Engine observers for the smalls DMA (so later ops carry only
            # their one data wait).
            dve_scratch = persist.tile([1, 1], F32, tag="dve_scratch")
            nc.vector.tensor_copy(out=dve_scratch, in_=sm[0:1, SM_BS : SM_BS + 1])
            uu_psum = ps_u.tile([1, C], F32, tag="uu_psum")
            nc.tensor.matmul(
                uu_psum[0:1, 0:1],
                lhsT=smbf(0, P, SM_WSBF, SM_WSBF + 1)[:, 0:1],
                rhs=smbf(0, P, SM_WSBF, SM_WSBF + 1)[:, 0:1],
                start=True,
                stop=True,
            )

            # ---- pass 1: per-chunk cast / rowsum / matvec / relu,
            # software-pipelined so the in-DMA-paced ops never sit behind
            # a cross-engine-dependent op on the same engine.
            s_ps = []

            def cast_chunk(ci):
                # cast f32->bf16 for the PE matvec; accum_out gives the
                # per-partition row-sum partial for free in the same op
                c0, w = CHUNKS[ci]
                nc.vector.tensor_scalar(
                    out=xbf[:, 0, c0 : c0 + w],
                    in0=xall[:, 0, c0 : c0 + w],
                    scalar1=0.0,
                    scalar2=0.0,
                    op0=ADD,
                    op1=ADD,
                    accum_out=xs_part[0][:, ci : ci + 1],
                )
                nc.scalar.activation(
                    out=xbf[:, 1, c0 : c0 + w],
                    in_=xall[:, 1, c0 : c0 + w],
                    func=mybir.ActivationFunctionType.Copy,
                    accum_out=xs_part[1][:, ci : ci + 1],
                )

            def matvec_chunk(ci):
                c0, w = CHUNKS[ci]
                ps = ps_s.tile([1, CW], F32, tag="s_psum")
                s_ps.append(ps)
                for t in range(NT):
                    nc.tensor.matmul(
                        ps[:, :w],
                        lhsT=smbf(0, P, SM_WSBF + t, SM_WSBF + t + 1)[:, 0:1],
                        rhs=xbf[:, t, c0 : c0 + w],
                        start=(t == 0),
                        stop=(t == NT - 1),
                    )

            def relu_chunk(ci):
                c0, w = CHUNKS[ci]
                nc.vector.tensor_scalar(
                    out=s_bf[0:1, c0 : c0 + w],
                    in0=s_ps[ci][0:1, :w],
                    scalar1=sm[0:1, SM_BS : SM_BS + 1],
                    scalar2=0.0,
                    op0=ADD,
                    op1=MAX,
                )

            cast_chunk(0)
            matvec_chunk(0)
            for ci in range(1, NCH):
                cast_chunk(ci)
                matvec_chunk(ci)
                relu_chunk(ci - 1)
            relu_chunk(NCH - 1)

            # ---- uu chain (after last chunk's rowsum partials).  All on
            # DVE after relu(last) in program order, so the uu matmul's
            # single DVE wait transitively covers every s_bf write too.
            for t in range(NT):
                nc.vector.reduce_sum(
                    out=xsum_sb[:, t : t + 1],
                    in_=xs_part[t][:, :],
                    axis=mybir.AxisListType.X,
                )
            nc.vector.tensor_copy(out=xsum_bf[:, :], in_=xsum_sb[:, :])

            # uu = Wve/N @ xsum + Wexp bv  (row layout [1, C], bf16 matmuls)
            nc.tensor.matmul(
                uu_psum[:, :],
                lhsT=smbf(0, 1, SM_ONEBF, SM_ONEBF + 1)[:, 0:1],
                rhs=smbf(0, 1, SM_WBVBF, SM_WBVBF + C // 2),
                start=True,
                stop=False,
                skip_group_check=True,
            )
            for t in range(NT):
                nc.tensor.matmul(
                    uu_psum[:, :],
                    lhsT=xsum_bf[:, t : t + 1],
                    rhs=smbf(0, P, SM_WVETBF + t * P, SM_WVETBF + (t + 1) * P),
                    start=False,
                    stop=(t == NT - 1),
                    skip_group_check=True,
                )
            nc.vector.tensor_copy(out=uu_bf[:, :], in_=uu_psum[:, :])

            # ---- pass 2: outer product (PE) -> psum copy (ACT) -> +x,+bexp
            # (DVE stt) -> chunk-pair output DMA (sync/scalar alternating)
            def outer_chunk(ci):
                # per-(chunk, tile) psum tiles, one matmul + one ACT copy
                # + one DVE stt each: keeps every hazard chain (WAW on the
                # psum bank, writer tracking on o_sb) exactly replaceable
                # so the framework's transitive wait elision kicks in
                c0, w = CHUNKS[ci]
                for t in range(NT):
                    po = ps_o.tile([P, CW], F32, tag="o_psum")
                    nc.tensor.matmul(
                        po[:, :w],
                        lhsT=uu_bf[0:1, t * P : (t + 1) * P],
                        rhs=s_bf[0:1, c0 : c0 + w],
                        start=True,
                        stop=True,
                    )
                    nc.vector.scalar_tensor_tensor(
                        out=o_sb[:, t, c0 : c0 + w],
                        in0=xall[:, t, c0 : c0 + w],
                        scalar=sm[:, SM_BEXPC + t : SM_BEXPC + t + 1],
                        in1=po[:, :w],
                        op0=ADD,
                        op1=ADD,
                    )

            oc = 0
            for ci in range(NCH):
                outer_chunk(ci)
                while oc < len(OCHUNKS) and OCHUNKS[oc][0] + OCHUNKS[oc][1] <= (
                    CHUNKS[ci][0] + CHUNKS[ci][1]
                ):
                    c0, w = OCHUNKS[oc]
                    eng = nc.sync if oc % 2 == 0 else nc.scalar
                    eng.dma_start(
                        out=out[:, c0 : c0 + w].rearrange(
                            "(t p) n -> p t n", p=P
                        ),
                        in_=o_sb[:, :, c0 : c0 + w],
                    )
                    oc += 1
            assert oc == len(OCHUNKS)
    return nc


def _pack_smalls(Wq, bq, Wk, bk, Wv, bv, Wcat, Wexp, bexp):
    import ml_dtypes

    f32 = np.float32
    wq_c, wk_c = Wcat[0, :E], Wcat[0, E:]
    wS = (Wq.T @ wq_c + Wk.T @ wk_c).astype(f32)  # [C]
    bS = f32(wq_c @ bq + wk_c @ bk)
    Wve = (Wexp @ Wv).astype(f32)  # [C, C]
    wvet = (Wve.T / f32(N)).astype(f32)  # [C, C] : [k, m]
    wexpbv = (Wexp @ bv).astype(f32)

    def bf(x):
        return np.asarray(x, f32).astype(ml_dtypes.bfloat16).view(np.uint16)

    sm = np.zeros((P, SM_F), f32)
    u16 = sm.view(np.uint16).reshape(P, SM_F, 2)  # little-endian halves
    for t in range(NT):
        u16[:, SM_WSBF + t, 0] = bf(wS[t * P : (t + 1) * P])
        # wvet[t] is [k=128, m=256] -> 128 f32 cols of 2 bf16 each
        u16[:, SM_WVETBF + t * P : SM_WVETBF + (t + 1) * P, :] = bf(
            wvet[t * P : (t + 1) * P, :]
        ).reshape(P, P, 2)
        sm[:, SM_BEXPC + t] = bexp[t * P : (t + 1) * P]
    u16[0, SM_ONEBF, 0] = bf(1.0)
    u16[0, SM_WBVBF : SM_WBVBF + C // 2, :] = bf(wexpbv).reshape(C // 2, 2)
    sm[0, SM_BS] = bS
    return sm


def kernel(x, Wq, bq, Wk, bk, Wv, bv, Wcat, Wexp, bexp):
    global LAST_RESULTS
    f32 = np.float32
    x = np.ascontiguousarray(np.asarray(x, f32))
    args = [np.asarray(a, f32) for a in (Wq, bq, Wk, bk, Wv, bv, Wcat, Wexp, bexp)]
    sm = _pack_smalls(*args)

    if "prog" not in _prog_cache:
        _prog_cache["prog"] = _build()
    nc = _prog_cache["prog"]

    xf = x.reshape(B, C, N)
    in_maps = [
        {"xb": np.ascontiguousarray(xf[b]), "smalls": sm} for b in range(B)
    ]

    LAST_RESULTS = run_bass_kernel_spmd(nc, in_maps, core_ids=list(range(B)))
    out = np.stack([LAST_RESULTS.results[b]["out"] for b in range(B)], axis=0)
    return out.reshape(B, C, H, W).astype(f32)


if __name__ == "__main__":
    rng = np.random.default_rng(0)
    s = 0.02
    f32 = np.float32
    args = dict(
        x=rng.standard_normal((B, C, H, W)).astype(f32),
        Wq=(rng.standard_normal((E, C)) * s).astype(f32),
        bq=(rng.standard_normal((E,)) * s).astype(f32),
        Wk=(rng.standard_normal((E, C)) * s).astype(f32),
        bk=(rng.standard_normal((E,)) * s).astype(f32),
        Wv=(rng.standard_normal((E, C)) * s).astype(f32),
        bv=(rng.standard_normal((E,)) * s).astype(f32),
        Wcat=(rng.standard_normal((1, 2 * E)) * s).astype(f32),
        Wexp=(rng.standard_normal((C, E)) * s).astype(f32),
        bexp=(rng.standard_normal((C,)) * s).astype(f32),
    )
    o = kernel(**args)
    print(o.shape, o.dtype)
